# revision 1
# baseline (speedup 1.0000x reference)
"""ROI-Align + MLP classification head (nms_detection) on 8 Trainium2 cores.

Strategy: data-parallel over batch (2 images per core). Per core, the kernel
computes bilinear sample coordinates from the proposals on-device, gathers
only the needed feature-map pixel pairs with indirect DMAs (~3 MB instead of
streaming the full 32 MB shard, cast to fp16 in-flight), does the bilinear
combine on the vector engine in fp16, transposes sample-major ->
feature-major on the PE, and runs the 3-layer MLP (fp16 in / fp32 psum) +
fp32 softmax.

Layouts (per core): 44 rois x 16 bin-centers = 704 samples.
  roi slot (h, g): roi = h*6 + g, h in 0..7, g in 0..5 (48 slots, 4 dup/garbage)
  sample partition p = h*16 + q (q = iy*4+ix), sample group = g.
  gather block j = ab*6 + g (ab = y-corner row 0/1), one indirect DMA each:
    G[p, j*512 :+512] = fm row pair (y0+ab, x0..x0+1) channels (512 floats).
"""

import numpy as np

import concourse.bacc as bacc
import concourse.bass as bass
import concourse.mybir as mybir
import concourse.tile as tile
from concourse._compat import get_trn_type
from concourse.bass_utils import run_bass_kernel_spmd

# Problem shape (hardcoded per contract)
B, P, H, W, C = 16, 22, 128, 128, 256
NUM_CLASSES = 10
N_CORES = 8
B_LOC = B // N_CORES        # 2 images per core
NROI = B_LOC * P            # 44 rois per core
NRS = 48                    # roi slots (8 partition-blocks x 6 groups)
NG = 6                      # sample groups of 128
HID1, HID2 = 128, 64
F32 = mybir.dt.float32
F16 = mybir.dt.float16
I32 = mybir.dt.int32
AX_X = mybir.AxisListType.X
OP = mybir.AluOpType
AF = mybir.ActivationFunctionType

NPIX = B_LOC * H * W            # 32768 flat pixel rows per core
MAX_ROW_A = NPIX - 130          # room for +1 col pair and +W row
MAGIC = 12582912.0              # 1.5 * 2^23 fp32 round-to-int magic


def _static_consts():
    ident = np.eye(128).astype(np.float16)
    p = np.arange(128)
    q = p % 16
    cy = ((q // 4).astype(np.float32) + 0.5) / 4.0
    cx = ((q % 4).astype(np.float32) + 0.5) / 4.0
    # per-sample batch offset: bofs[p, g] for roi = (p//16)*6 + g
    h = np.arange(128)[:, None] // 16
    g = np.arange(NG)[None, :]
    roi = h * 6 + g                                               # [128, 6]
    bofs = np.where(roi >= P, float(H * W), 0.0).astype(np.float32)
    cb32 = np.concatenate([cy[:, None], cx[:, None], bofs], axis=1).astype(np.float32)
    cidx = np.minimum(np.arange(128) // 16 * 6, 38).astype(np.int32)[:, None]  # [128,1]
    return ident, cb32, cidx


def emit_kernel(nc, tc, fm, prop, W1, b1, W2, b2, W3, b3, out, consts):
    """Emit the per-core tile kernel. All args are bass.APs."""
    with (
        tc.tile_pool(name="const", bufs=1) as cpool,
        tc.tile_pool(name="work", bufs=1) as wpool,
        tc.tile_pool(name="psum", bufs=1, space="PSUM") as ppool,
    ):
        _emit_body(nc, tc, fm, prop, W1, b1, W2, b2, W3, b3, out, consts,
                   cpool, wpool, ppool)


def _emit_body(nc, tc, fm, prop, W1, b1, W2, b2, W3, b3, out, consts,
               cpool, wpool, ppool):
    ident_c, cb32_c, cidx_c = consts
    V = nc.vector

    # ---------------- bundled consts (3 small DMAs) ----------------
    ident = cpool.tile([128, 128], F16, name="ident")
    nc.sync.dma_start(ident[:], ident_c)
    cb32 = cpool.tile([128, 8], F32, name="cb32")
    nc.sync.dma_start(cb32[:], cb32_c)
    cidx = cpool.tile([128, 1], I32, name="cidx")
    nc.sync.dma_start(cidx[:], cidx_c)
    cy_ap, cx_ap, bofs = cb32[:, 0:1], cb32[:, 1:2], cb32[:, 2:8]

    # ---------------- coords expansion: one POOL gather + fixup ----------
    # Ct[p, g*4+k] = proposals[roi(p//16, g), k]; h=7 block reads rois 38..43
    Ct = cpool.tile([128, NG * 4], F32, name="coords")
    pv = prop.rearrange("b p k -> (b p) k")                       # [44, 4]
    nc.gpsimd.indirect_dma_start(
        out=Ct[:], out_offset=None, in_=pv,
        in_offset=bass.IndirectOffsetOnAxis(ap=cidx[:, :], axis=0))
    # h=7 fixup: slots (g=0,1) must hold rois 42,43
    nc.sync.dma_start(Ct[112:128, 0:8], Ct[112:128, 16:24])

    cgv = Ct[:, :].rearrange("p (g k) -> p g k", g=NG)
    y1c, x1c, y2c, x2c = (cgv[:, :, k] for k in range(4))

    # ---------------- index chain (critical path to the gathers) --------
    def t6(name):
        return wpool.tile([128, NG], F32, name=name)

    dy, ys, dx, xs = t6("dy"), t6("ys"), t6("dx"), t6("xs")
    ly, y0f, lx, x0f = t6("ly"), t6("y0f"), t6("lx"), t6("x0f")
    hy, hx, pixf = t6("hy"), t6("hx"), t6("pixf")

    V.tensor_tensor(out=dy[:], in0=y2c, in1=y1c, op=OP.subtract)
    V.tensor_scalar(out=ys[:], in0=dy[:], scalar1=cy_ap, scalar2=None, op0=OP.mult)
    V.tensor_tensor(out=ys[:], in0=ys[:], in1=y1c, op=OP.add)
    V.tensor_tensor(out=dx[:], in0=x2c, in1=x1c, op=OP.subtract)
    V.tensor_scalar(out=xs[:], in0=dx[:], scalar1=cx_ap, scalar2=None, op0=OP.mult)
    V.tensor_tensor(out=xs[:], in0=xs[:], in1=x1c, op=OP.add)
    # y0 = round(ys - 0.5) via fp32 magic; consistent-pair bilinear stays exact
    V.tensor_scalar(out=y0f[:], in0=ys[:], scalar1=-0.5, scalar2=MAGIC, op0=OP.add, op1=OP.add)
    V.tensor_scalar(out=y0f[:], in0=y0f[:], scalar1=-MAGIC, scalar2=None, op0=OP.add)
    V.tensor_scalar(out=x0f[:], in0=xs[:], scalar1=-0.5, scalar2=MAGIC, op0=OP.add, op1=OP.add)
    V.tensor_scalar(out=x0f[:], in0=x0f[:], scalar1=-MAGIC, scalar2=None, op0=OP.add)
    # pix = b*H*W + y0*W + x0, clamped
    V.tensor_scalar(out=pixf[:], in0=y0f[:], scalar1=float(W), scalar2=None, op0=OP.mult)
    V.tensor_tensor(out=pixf[:], in0=pixf[:], in1=x0f[:], op=OP.add)
    V.tensor_tensor(out=pixf[:], in0=pixf[:], in1=bofs, op=OP.add)
    V.tensor_scalar(out=pixf[:], in0=pixf[:], scalar1=0.0, scalar2=float(MAX_ROW_A),
                    op0=OP.max, op1=OP.min)
    idx = cpool.tile([128, 12], I32, name="gidx")
    V.tensor_copy(out=idx[:, 0:NG], in_=pixf[:])
    V.tensor_scalar(out=idx[:, NG:12], in0=pixf[:], scalar1=float(W), scalar2=None, op0=OP.add)

    # ---------------- gather: 12 indirect DMAs (fp16 cast in-flight) ------
    G = wpool.tile([128, 12 * 512], F16, name="gather")
    fmv = fm.rearrange("b h w c -> (b h w) c")                    # [32768, 256]
    half = NG // 2
    CHUNK_J = [(0, 1, 2, 6, 7, 8), (3, 4, 5, 9, 10, 11)]
    for js in CHUNK_J:
        for j in js:
            nc.gpsimd.indirect_dma_start(
                out=G[:, j * 512:(j + 1) * 512],
                out_offset=None,
                in_=fmv,
                in_offset=bass.IndirectOffsetOnAxis(ap=idx[:, j:j + 1], axis=0),
            )

    # bilinear corner weights (off the gather critical path), fp16
    V.tensor_tensor(out=ly[:], in0=ys[:], in1=y0f[:], op=OP.subtract)
    V.tensor_tensor(out=lx[:], in0=xs[:], in1=x0f[:], op=OP.subtract)
    V.tensor_scalar(out=hy[:], in0=ly[:], scalar1=-1.0, scalar2=1.0, op0=OP.mult, op1=OP.add)
    V.tensor_scalar(out=hx[:], in0=lx[:], scalar1=-1.0, scalar2=1.0, op0=OP.mult, op1=OP.add)
    wc = cpool.tile([128, 24], F16, name="wcat")   # free = (ab, g, xc)
    wv = wc[:, :].rearrange("p (ab g x) -> p ab g x", ab=2, x=2)
    V.tensor_tensor(out=wv[:, 0, :, 0], in0=hy[:], in1=hx[:], op=OP.mult)
    V.tensor_tensor(out=wv[:, 0, :, 1], in0=hy[:], in1=lx[:], op=OP.mult)
    V.tensor_tensor(out=wv[:, 1, :, 0], in0=ly[:], in1=hx[:], op=OP.mult)
    V.tensor_tensor(out=wv[:, 1, :, 1], in0=ly[:], in1=lx[:], op=OP.mult)

    # ---------------- weight/bias loads (overlap the gather) --------------
    W1f = cpool.tile([128, 4096], F32, name="W1f")
    nc.sync.dma_start(W1f[:, :].rearrange("p (k h) -> p k h", k=32),
                      W1.rearrange("(k p) h -> p k h", p=128))
    W1sb = cpool.tile([128, 4096], F16, name="W1sb")
    nc.scalar.copy(out=W1sb[:], in_=W1f[:])
    W2f = cpool.tile([128, HID2], F32, name="W2f")
    nc.sync.dma_start(W2f[:], W2)
    W2sb = cpool.tile([128, HID2], F16, name="W2sb")
    nc.scalar.copy(out=W2sb[:], in_=W2f[:])
    W3f = cpool.tile([HID2, NUM_CLASSES], F32, name="W3f")
    nc.sync.dma_start(W3f[:], W3)
    W3sb = cpool.tile([HID2, NUM_CLASSES], F16, name="W3sb")
    nc.scalar.copy(out=W3sb[:], in_=W3f[:])
    b1sb = cpool.tile([128, 1], F32, name="b1sb")
    nc.sync.dma_start(b1sb[:], b1.rearrange("(p o) -> p o", o=1))
    b2sb = cpool.tile([HID2, 1], F32, name="b2sb")
    nc.sync.dma_start(b2sb[:], b2.rearrange("(p o) -> p o", o=1))
    b3sb = cpool.tile([NROI, NUM_CLASSES], F32, name="b3sb")
    nc.sync.dma_start(b3sb[:], b3.unsqueeze(0).to_broadcast([NROI, NUM_CLASSES]))

    # ---------------- bilinear combine + transpose, 2 group-chunks --------
    Gv = G[:, :].rearrange("p (ab g x c) -> p ab g x c", ab=2, g=NG, x=2)
    wb = wc[:, :].rearrange("p (ab g x) -> p ab g x", ab=2, x=2).unsqueeze(4) \
        .to_broadcast([128, 2, NG, 2, C])
    sv2 = wpool.tile([128, NG * 512], F16, name="sv2")
    sv = wpool.tile([128, NG * 256], F16, name="sv")
    s2v = sv2[:, :].rearrange("p (g x c) -> p g x c", g=NG, x=2)
    svv = sv[:, :].rearrange("p (g c) -> p g c", g=NG)
    svT = [wpool.tile([128, NG * 128], F16, name=f"svT{h}") for h in range(2)]
    for cix in range(2):
        gs = slice(cix * half, (cix + 1) * half)
        V.tensor_tensor(out=Gv[:, :, gs, :, :], in0=Gv[:, :, gs, :, :],
                        in1=wb[:, :, gs, :, :], op=OP.mult)
        V.tensor_tensor(out=sv2[:, cix * 1536:(cix + 1) * 1536],
                        in0=G[:, cix * 1536:cix * 1536 + 1536],
                        in1=G[:, 3072 + cix * 1536:3072 + cix * 1536 + 1536],
                        op=OP.add)
        V.tensor_tensor(out=svv[:, gs, :], in0=s2v[:, gs, 0, :], in1=s2v[:, gs, 1, :],
                        op=OP.add)
        for h in range(2):
            for g in range(cix * half, (cix + 1) * half):
                pt = ppool.tile([128, 128], F16, tag="pt", bufs=4, name="pt")
                nc.tensor.transpose(out=pt[:],
                                    in_=sv[:, g * 256 + h * 128: g * 256 + (h + 1) * 128],
                                    identity=ident[:])
                nc.scalar.copy(out=svT[h][:, g * 128:(g + 1) * 128], in_=pt[:])

    # ---------------- MLP ----------------
    # psum1 columns j = a*6 + b = roi (a = h in 0..7, b = g in 0..5)
    psum1 = ppool.tile([128, NRS], F32, name="psum1")
    for h in range(2):
        for q in range(16):
            k = q * 2 + h
            rhs = svT[h][:, :].rearrange("p (b a s) -> p a b s", b=6, a=8)[:, :, :, q]
            nc.tensor.matmul(out=psum1[:], lhsT=W1sb[:, k * 128:(k + 1) * 128], rhs=rhs,
                             start=(h == 0 and q == 0), stop=(h == 1 and q == 15))
    l1 = wpool.tile([128, NRS], F16, name="l1")
    nc.scalar.activation(out=l1[:], in_=psum1[:], func=AF.Relu, bias=b1sb[:, 0:1], scale=1.0)

    psum2 = ppool.tile([HID2, NRS], F32, name="psum2")
    nc.tensor.matmul(out=psum2[:], lhsT=W2sb[:, :], rhs=l1[:], start=True, stop=True)
    l2 = wpool.tile([HID2, NRS], F16, name="l2")
    nc.scalar.activation(out=l2[:], in_=psum2[:], func=AF.Relu, bias=b2sb[:, 0:1], scale=1.0)

    psum3 = ppool.tile([NRS, NUM_CLASSES], F32, name="psum3")
    nc.tensor.matmul(out=psum3[:], lhsT=l2[:], rhs=W3sb[:], start=True, stop=True)

    # ---------------- softmax (rows 0..43 only, fp32) ----------------
    logits = wpool.tile([NROI, NUM_CLASSES], F32, name="logits")
    V.tensor_tensor(out=logits[:], in0=psum3[0:NROI, :], in1=b3sb[:], op=OP.add)
    mxn = wpool.tile([NROI, 1], F32, name="mxn")
    V.tensor_reduce(out=mxn[:], in_=logits[:], axis=AX_X, op=OP.max, negate=True)
    ex = wpool.tile([NROI, NUM_CLASSES], F32, name="ex")
    nc.scalar.activation(out=ex[:], in_=logits[:], func=AF.Exp, bias=mxn[:, 0:1], scale=1.0)
    ssum = wpool.tile([NROI, 1], F32, name="ssum")
    V.tensor_reduce(out=ssum[:], in_=ex[:], axis=AX_X, op=OP.add)
    rinv = wpool.tile([NROI, 1], F32, name="rinv")
    V.reciprocal(rinv[:], ssum[:])
    probs = wpool.tile([NROI, NUM_CLASSES], F32, name="probs")
    V.tensor_scalar(out=probs[:], in0=ex[:], scalar1=rinv[:, 0:1], scalar2=None, op0=OP.mult)

    nc.sync.dma_start(out.rearrange("b p c -> (b p) c"), probs[:])


def build_module():
    nc = bacc.Bacc(get_trn_type() or "TRN2", target_bir_lowering=False, debug=False)
    fm = nc.dram_tensor("feature_map", [B_LOC, H, W, C], F32, kind="ExternalInput")
    prop = nc.dram_tensor("proposals", [B_LOC, P, 4], F32, kind="ExternalInput")
    W1 = nc.dram_tensor("W1", [4096, HID1], F32, kind="ExternalInput")
    b1 = nc.dram_tensor("b1", [HID1], F32, kind="ExternalInput")
    W2 = nc.dram_tensor("W2", [HID1, HID2], F32, kind="ExternalInput")
    b2 = nc.dram_tensor("b2", [HID2], F32, kind="ExternalInput")
    W3 = nc.dram_tensor("W3", [HID2, NUM_CLASSES], F32, kind="ExternalInput")
    b3 = nc.dram_tensor("b3", [NUM_CLASSES], F32, kind="ExternalInput")
    out = nc.dram_tensor("out", [B_LOC, P, NUM_CLASSES], F32, kind="ExternalOutput")

    ident_np, cb32_np, cidx_np = _static_consts()
    ident_c = nc.inline_tensor(ident_np, name="c_ident")
    cb32_c = nc.inline_tensor(cb32_np, name="c_cb32")
    cidx_c = nc.inline_tensor(cidx_np, name="c_cidx")

    with tile.TileContext(nc) as tc:
        emit_kernel(nc, tc, fm[:], prop[:], W1[:], b1[:], W2[:], b2[:], W3[:], b3[:],
                    out[:], (ident_c[:], cb32_c[:], cidx_c[:]))
    nc.compile()
    return nc


_NC_CACHE = None


def _get_module():
    global _NC_CACHE
    if _NC_CACHE is None:
        _NC_CACHE = build_module()
    return _NC_CACHE


def _shard_inputs(inputs):
    f = {k: np.ascontiguousarray(np.asarray(v, dtype=np.float32)) for k, v in inputs.items()}
    in_maps = []
    for c in range(N_CORES):
        sl = slice(B_LOC * c, B_LOC * (c + 1))
        in_maps.append({
            "feature_map": f["feature_map"][sl],
            "proposals": f["proposals"][sl],
            "W1": f["W1"], "b1": f["b1"],
            "W2": f["W2"], "b2": f["b2"],
            "W3": f["W3"], "b3": f["b3"],
        })
    return in_maps


def run(inputs, trace=False):
    """Run on all 8 cores; returns (output [16,22,10], BassKernelResults)."""
    nc = _get_module()
    res = run_bass_kernel_spmd(nc, _shard_inputs(inputs), core_ids=list(range(N_CORES)),
                               trace=trace)
    out = np.concatenate([r["out"] for r in res.results], axis=0)
    return out, res


def kernel(**inputs) -> np.ndarray:
    out, _ = run(inputs, trace=False)
    return out



# revision 16
# speedup vs baseline: 1.1917x; 1.1917x over previous
"""ROI-Align + MLP classification head (nms_detection) on 8 Trainium2 cores.

Strategy: data-parallel over batch (2 images per core). Per core:
  - proposals broadcast to a block-layout coord tile (3 leading-broadcast
    DMAs; every partition holds all 48 roi-slots' coords),
  - a DVE chain computes int16 feature-row indices for all 4 bilinear
    corners in the 16-partition-wrapped layout dma_gather wants (the
    chain runs on all 128 partitions so the pattern lands replicated
    across the 8 gpsimd cores),
  - the pixel fetch runs as 3 chunked dma_gather ops (InstDMAGatherAnt,
    512 true indices each, elem_step=256 overlapping row-pair windows)
    instead of 12 serialized 1-offset INDIRECT1Ds,
  - bilinear corner weights are computed in sample layout from a second
    (sample-layout) coord broadcast,
  - bilinear combine on DVE per chunk (fp32 gather x fp32 weight ->
    fp16, fusing the cast), PE transpose to feature-major, 3-layer MLP
    (fp16 in / fp32 psum) + fp32 softmax.

Layouts (per core): 44 rois x 16 bin-centers = 704 samples.
  partition p = hb*16 + q (hb 0..7, q = iy*4+ix), image b = hb//4,
  roi slot (hb, g): roi row R0[hb] + g; psum column j = hb*6 + g; the
  output DMA picks rows j to match.
  gather chunk c (g-pair): G[p, (c, ab, g2, x, ch)]; index list position
  i = (ab*2+g2)*128 + p per chunk -> idxs[i%16, c*32 + i//16].
"""

import numpy as np

import concourse.bacc as bacc
import concourse.bass as bass
import concourse.mybir as mybir
import concourse.tile as tile
from concourse._compat import get_trn_type
from concourse.bass_utils import run_bass_kernel_spmd

# Problem shape (hardcoded per contract)
B, P, H, W, C = 16, 22, 128, 128, 256
NUM_CLASSES = 10
N_CORES = 8
B_LOC = B // N_CORES        # 2 images per core
NROI = B_LOC * P            # 44 rois per core
NRS = 48                    # roi slots (8 partition-blocks x 6 groups)
NG = 6                      # sample groups
NCHUNK = 3                  # gather chunks (g-pairs)
HID1, HID2 = 128, 64
F32 = mybir.dt.float32
F16 = mybir.dt.float16
I16 = mybir.dt.int16
AX_X = mybir.AxisListType.X
OP = mybir.AluOpType
AF = mybir.ActivationFunctionType

NPIX = B_LOC * H * W            # 32768 flat pixel rows per core
MAGIC = 12582912.0              # 1.5 * 2^23 fp32 round-to-int magic


def _static_consts():
    ident = np.eye(128).astype(np.float16)
    p = np.arange(128)
    q = p % 16
    cy = ((q // 4).astype(np.float32) + 0.5) / 4.0
    cx = ((q % 4).astype(np.float32) + 0.5) / 4.0
    # batch offset by roi-slot column (hb, g): H*W*(hb//4), same on all rows
    bf = np.repeat((np.arange(8) // 4).astype(np.float32) * float(H * W), NG)[None, :]
    bf = np.broadcast_to(bf, (128, NRS))
    cb = np.concatenate([cy[:, None], cx[:, None], bf], axis=1).astype(np.float32)
    return ident, cb                                              # [128, 50]


def emit_kernel(nc, tc, fm, prop, W1, b1, W2, b2, W3, b3, out, consts):
    """Emit the per-core tile kernel. All args are bass.APs."""
    with (
        tc.tile_pool(name="const", bufs=1) as cpool,
        tc.tile_pool(name="work", bufs=1) as wpool,
        tc.tile_pool(name="psum", bufs=1, space="PSUM") as ppool,
    ):
        _emit_body(nc, tc, fm, prop, W1, b1, W2, b2, W3, b3, out, consts,
                   cpool, wpool, ppool)


def _emit_body(nc, tc, fm, prop, W1, b1, W2, b2, W3, b3, out, consts,
               cpool, wpool, ppool):
    ident_c, cb_c = consts
    V = nc.vector

    # ---------------- consts ----------------
    cb = cpool.tile([128, 2 + NRS], F32, name="cb")
    nc.sync.dma_start(cb[:], cb_c)
    cy_ap, cx_ap, bf48 = cb[:, 0:1], cb[:, 1:2], cb[:, 2:2 + NRS]

    # ---------------- coords, block layout -------------------------------
    # Ct48[p, (hb, g, k)] = prop_flat[R0[hb] + g, k] on every partition,
    # R0 = [0, 6, 12, 18, 22, 28, 34, 38] ((b=1,hl=3) clamped to 38 so the
    # read stays in bounds; that block holds rois 38..43 and the output
    # DMA picks matching rows). Source runs are contiguous; leading-dim
    # (partition) broadcast only -- middle-dim stride-0 mis-synchronizes.
    Ct48 = cpool.tile([128, NRS * 4], F32, name="coords48")
    pv = prop.rearrange("b p k -> (b p) k")                       # [44, 4]
    for cols, lo, hi in ((slice(0, 96), 0, 24), (slice(96, 168), 22, 40),
                         (slice(168, 192), 38, 44)):
        nc.sync.dma_start(
            Ct48[:, cols],
            pv[lo:hi, :].rearrange("r k -> (r k)")
            .unsqueeze(0).to_broadcast([128, (hi - lo) * 4]))

    cgv = Ct48[:, :].rearrange("p (s k) -> p s k", s=NRS)
    y1c, x1c, y2c, x2c = (cgv[:, :, k] for k in range(4))

    # sample-layout coords for the bilinear-weight chain:
    # Ct[p=(hb,q), (g,k)] = prop_flat[R0[hb]+g, k], leading-dim broadcast
    R0 = [0, 6, 12, 18, 22, 28, 34, 38]
    Ct = cpool.tile([128, NG * 4], F32, name="coords")
    for hb in range(8):
        nc.sync.dma_start(
            Ct[hb * 16:(hb + 1) * 16, :],
            pv[R0[hb]:R0[hb] + NG, :].rearrange("g k -> (g k)")
            .unsqueeze(0).to_broadcast([16, 24]))
    c6v = Ct[:, :].rearrange("p (g k) -> p g k", g=NG)
    y1s, x1s, y2s, x2s = (c6v[:, :, k] for k in range(4))

    # ---------------- W1 load (off the critical path) --------------------
    W1f = cpool.tile([128, 4096], F32, name="W1f")
    nc.sync.dma_start(W1f[:, :].rearrange("p (k h) -> p k h", k=32),
                      W1.rearrange("(k p) h -> p k h", p=128))
    ident = cpool.tile([128, 128], F16, name="ident")
    nc.sync.dma_start(ident[:], ident_c)

    # ---------------- index chain (critical path to the gathers) --------
    def t48(name):
        return wpool.tile([128, NRS], F32, name=name)

    dy, ys, dx, xs = t48("dy"), t48("ys"), t48("dx"), t48("xs")
    y0f, x0f, pixf = t48("y0f"), t48("x0f"), t48("pixf")

    V.tensor_tensor(out=dy[:], in0=y2c, in1=y1c, op=OP.subtract)
    V.tensor_scalar(out=ys[:], in0=dy[:], scalar1=cy_ap, scalar2=None, op0=OP.mult)
    V.tensor_tensor(out=ys[:], in0=ys[:], in1=y1c, op=OP.add)
    V.tensor_tensor(out=dx[:], in0=x2c, in1=x1c, op=OP.subtract)
    V.tensor_scalar(out=xs[:], in0=dx[:], scalar1=cx_ap, scalar2=None, op0=OP.mult)
    V.tensor_tensor(out=xs[:], in0=xs[:], in1=x1c, op=OP.add)
    # y0 = round(ys - 0.5) via fp32 magic; consistent-pair bilinear stays exact
    V.tensor_scalar(out=y0f[:], in0=ys[:], scalar1=-0.5, scalar2=MAGIC, op0=OP.add, op1=OP.add)
    V.tensor_scalar(out=y0f[:], in0=y0f[:], scalar1=-MAGIC, scalar2=None, op0=OP.add)
    V.tensor_scalar(out=x0f[:], in0=xs[:], scalar1=-0.5, scalar2=MAGIC, op0=OP.add, op1=OP.add)
    V.tensor_scalar(out=x0f[:], in0=x0f[:], scalar1=-MAGIC, scalar2=None, op0=OP.add)
    # pix = y0*W + b*H*W + x0; always a valid row (coords are pre-clipped)
    V.tensor_scalar(out=pixf[:], in0=y0f[:], scalar1=float(W), scalar2=None, op0=OP.mult)
    V.tensor_tensor(out=pixf[:], in0=pixf[:], in1=bf48, op=OP.add)
    V.tensor_tensor(out=pixf[:], in0=pixf[:], in1=x0f[:], op=OP.add)
    # int16 index tile, 16-partition-wrapped: idxs[p, c*32+(ab*2+g2)*8+hb];
    # the chain ran on all 128 partitions with p%16-periodic inputs, so the
    # pattern is already replicated for the 8 gpsimd cores.
    idxs = cpool.tile([128, 96], I16, name="idxs")
    idxv = idxs[:, :].rearrange("p (c ab g2 hb) -> p c ab g2 hb", c=NCHUNK, ab=2, g2=2)
    pixv = pixf[:, :].rearrange("p (hb c g2) -> p c g2 hb", hb=8, c=NCHUNK)
    V.tensor_copy(out=idxv[:, :, 0], in_=pixv)
    V.tensor_scalar(out=idxv[:, :, 1], in0=pixv, scalar1=float(W), scalar2=None, op0=OP.add)

    # ---------------- gather: 3 chunked dma_gathers (true indices) -------
    G = wpool.tile([128, 12 * 512], F32, name="gather")
    src = bass.AP(fm.tensor, 0, [[C, NPIX - 1], [1, 2 * C]])      # row-pair windows
    for c in range(NCHUNK):
        nc.gpsimd.dma_gather(
            out_ap=G[:, c * 2048:(c + 1) * 2048].rearrange("p (s e) -> p s e", s=4),
            in_ap=src,
            idxs_ap=idxs[:, c * 32:(c + 1) * 32],
            num_idxs=512,
            num_idxs_reg=512,
            elem_size=2 * C,
            elem_step=C,
        )

    # small weight/bias loads on Pool SWDGE (queue drains after gathers);
    # W2/W3 cast fp32 -> fp16 in flight (only gpsimd DMAs can cast)
    W2sb = cpool.tile([128, HID2], F16, name="W2sb")
    nc.gpsimd.dma_start(W2sb[:], W2)
    W3sb = cpool.tile([HID2, NUM_CLASSES], F16, name="W3sb")
    nc.gpsimd.dma_start(W3sb[:], W3)
    b1sb = cpool.tile([128, 1], F32, name="b1sb")
    nc.gpsimd.dma_start(b1sb[:], b1.rearrange("(p o) -> p o", o=1))
    b2sb = cpool.tile([HID2, 1], F32, name="b2sb")
    nc.gpsimd.dma_start(b2sb[:], b2.rearrange("(p o) -> p o", o=1))
    b3sb = cpool.tile([NRS, NUM_CLASSES], F32, name="b3sb")
    nc.gpsimd.dma_start(b3sb[:], b3.unsqueeze(0).to_broadcast([NRS, NUM_CLASSES]))

    # bilinear corner weights, sample layout (off the gather critical path)
    def t6(name):
        return wpool.tile([128, NG], F32, name=name)

    dy6, ys6, dx6, xs6 = t6("dy6"), t6("ys6"), t6("dx6"), t6("xs6")
    y0f6, x0f6, ly6, lx6 = t6("y0f6"), t6("x0f6"), t6("ly6"), t6("lx6")
    hy6, hx6 = t6("hy6"), t6("hx6")
    V.tensor_tensor(out=dy6[:], in0=y2s, in1=y1s, op=OP.subtract)
    V.tensor_scalar(out=ys6[:], in0=dy6[:], scalar1=cy_ap, scalar2=None, op0=OP.mult)
    V.tensor_tensor(out=ys6[:], in0=ys6[:], in1=y1s, op=OP.add)
    V.tensor_tensor(out=dx6[:], in0=x2s, in1=x1s, op=OP.subtract)
    V.tensor_scalar(out=xs6[:], in0=dx6[:], scalar1=cx_ap, scalar2=None, op0=OP.mult)
    V.tensor_tensor(out=xs6[:], in0=xs6[:], in1=x1s, op=OP.add)
    V.tensor_scalar(out=y0f6[:], in0=ys6[:], scalar1=-0.5, scalar2=MAGIC, op0=OP.add, op1=OP.add)
    V.tensor_scalar(out=y0f6[:], in0=y0f6[:], scalar1=-MAGIC, scalar2=None, op0=OP.add)
    V.tensor_scalar(out=x0f6[:], in0=xs6[:], scalar1=-0.5, scalar2=MAGIC, op0=OP.add, op1=OP.add)
    V.tensor_scalar(out=x0f6[:], in0=x0f6[:], scalar1=-MAGIC, scalar2=None, op0=OP.add)
    V.tensor_tensor(out=ly6[:], in0=ys6[:], in1=y0f6[:], op=OP.subtract)
    V.tensor_tensor(out=lx6[:], in0=xs6[:], in1=x0f6[:], op=OP.subtract)
    V.tensor_scalar(out=hy6[:], in0=ly6[:], scalar1=-1.0, scalar2=1.0, op0=OP.mult, op1=OP.add)
    V.tensor_scalar(out=hx6[:], in0=lx6[:], scalar1=-1.0, scalar2=1.0, op0=OP.mult, op1=OP.add)
    wc = cpool.tile([128, 24], F32, name="wcat")
    wcv = wc[:, :].rearrange("p (ab g x) -> p ab g x", ab=2, x=2)
    V.tensor_tensor(out=wcv[:, 0, :, 0], in0=hy6[:], in1=hx6[:], op=OP.mult)
    V.tensor_tensor(out=wcv[:, 0, :, 1], in0=hy6[:], in1=lx6[:], op=OP.mult)
    V.tensor_tensor(out=wcv[:, 1, :, 0], in0=ly6[:], in1=hx6[:], op=OP.mult)
    V.tensor_tensor(out=wcv[:, 1, :, 1], in0=ly6[:], in1=lx6[:], op=OP.mult)

    # W1 cast fp32 -> fp16 on the scalar engine, chunked (overlaps gather)
    W1sb = cpool.tile([128, 4096], F16, name="W1sb")
    for c in range(4):
        nc.scalar.copy(out=W1sb[:, c * 1024:(c + 1) * 1024],
                       in_=W1f[:, c * 1024:(c + 1) * 1024])

    # ---------------- bilinear combine + transpose, per chunk ------------
    # G chunk c: (ab, g2, x, ch) fp32; Gw: same, fp16 (cast fused into mult)
    Gv = G[:, :].rearrange("p (cc ab g x ch) -> p cc ab g x ch",
                           cc=NCHUNK, ab=2, g=2, x=2)
    Gw = wpool.tile([128, 12 * 512], F16, name="Gw")
    Gwv = Gw[:, :].rearrange("p (cc ab g x ch) -> p cc ab g x ch",
                             cc=NCHUNK, ab=2, g=2, x=2)
    sv2 = wpool.tile([128, NG * 512], F16, name="sv2")
    sv = wpool.tile([128, NG * 256], F16, name="sv")
    s2v = sv2[:, :].rearrange("p (g x ch) -> p g x ch", g=NG, x=2)
    svv = sv[:, :].rearrange("p (g ch) -> p g ch", g=NG)
    svT = [wpool.tile([128, NG * 128], F16, name=f"svT{h}") for h in range(2)]
    for c in range(NCHUNK):
        gs = slice(2 * c, 2 * c + 2)
        wb = wcv[:, :, gs, :].unsqueeze(4).to_broadcast([128, 2, 2, 2, C])
        V.tensor_tensor(out=Gwv[:, c], in0=Gv[:, c], in1=wb, op=OP.mult)
        V.tensor_tensor(out=s2v[:, gs], in0=Gwv[:, c, 0], in1=Gwv[:, c, 1], op=OP.add)
        V.tensor_tensor(out=svv[:, gs], in0=s2v[:, gs, 0, :], in1=s2v[:, gs, 1, :],
                        op=OP.add)
        for g in (2 * c, 2 * c + 1):
            for h in range(2):
                pt = ppool.tile([128, 128], F16, tag="pt", bufs=4, name="pt")
                nc.tensor.transpose(out=pt[:],
                                    in_=sv[:, g * 256 + h * 128: g * 256 + (h + 1) * 128],
                                    identity=ident[:])
                nc.scalar.copy(out=svT[h][:, g * 128:(g + 1) * 128], in_=pt[:])

    # ---------------- MLP ----------------
    # psum1 columns j = hb*6 + g (roi slot)
    psum1 = ppool.tile([128, NRS], F32, name="psum1")
    for k in range(32):
        q, h = k // 2, k % 2
        rhs = svT[h][:, :].rearrange("ch (g hb q) -> ch q hb g", g=NG, hb=8)[:, q]
        nc.tensor.matmul(out=psum1[:], lhsT=W1sb[:, k * 128:(k + 1) * 128], rhs=rhs,
                         start=(k == 0), stop=(k == 31))
    l1 = wpool.tile([128, NRS], F16, name="l1")
    nc.scalar.activation(out=l1[:], in_=psum1[:], func=AF.Relu, bias=b1sb[:, 0:1], scale=1.0)

    psum2 = ppool.tile([HID2, NRS], F32, name="psum2")
    nc.tensor.matmul(out=psum2[:], lhsT=W2sb[:, :], rhs=l1[:], start=True, stop=True)
    l2 = wpool.tile([HID2, NRS], F16, name="l2")
    nc.scalar.activation(out=l2[:], in_=psum2[:], func=AF.Relu, bias=b2sb[:, 0:1], scale=1.0)

    psum3 = ppool.tile([NRS, NUM_CLASSES], F32, name="psum3")
    nc.tensor.matmul(out=psum3[:], lhsT=l2[:], rhs=W3sb[:], start=True, stop=True)

    # ---------------- softmax (all 48 slots, pads discarded on store) ----
    logits = wpool.tile([NRS, NUM_CLASSES], F32, name="logits")
    V.tensor_tensor(out=logits[:], in0=psum3[:, :], in1=b3sb[:], op=OP.add)
    mxn = wpool.tile([NRS, 1], F32, name="mxn")
    V.tensor_reduce(out=mxn[:], in_=logits[:], axis=AX_X, op=OP.max, negate=True)
    ex = wpool.tile([NRS, NUM_CLASSES], F32, name="ex")
    nc.scalar.activation(out=ex[:], in_=logits[:], func=AF.Exp, bias=mxn[:, 0:1], scale=1.0)
    ssum = wpool.tile([NRS, 1], F32, name="ssum")
    V.tensor_reduce(out=ssum[:], in_=ex[:], axis=AX_X, op=OP.add)
    rinv = wpool.tile([NRS, 1], F32, name="rinv")
    V.reciprocal(rinv[:], ssum[:])
    probs = wpool.tile([NRS, NUM_CLASSES], F32, name="probs")
    V.tensor_scalar(out=probs[:], in0=ex[:], scalar1=rinv[:, 0:1], scalar2=None, op0=OP.mult)

    # slot j -> roi: img0 j 0..21 <-> r 0..21; img1 j 24..39 <-> r 0..15,
    # j 42..47 <-> r 16..21 (the hb=7 block holds rois 38..43)
    ov = out.rearrange("b p c -> (b p) c")
    nc.sync.dma_start(ov[0:22, :], probs[0:22, :])
    nc.sync.dma_start(ov[22:38, :], probs[24:40, :])
    nc.sync.dma_start(ov[38:44, :], probs[42:48, :])


def build_module():
    nc = bacc.Bacc(get_trn_type() or "TRN2", target_bir_lowering=False, debug=False)
    fm = nc.dram_tensor("feature_map", [B_LOC, H, W, C], F32, kind="ExternalInput")
    prop = nc.dram_tensor("proposals", [B_LOC, P, 4], F32, kind="ExternalInput")
    W1 = nc.dram_tensor("W1", [4096, HID1], F32, kind="ExternalInput")
    b1 = nc.dram_tensor("b1", [HID1], F32, kind="ExternalInput")
    W2 = nc.dram_tensor("W2", [HID1, HID2], F32, kind="ExternalInput")
    b2 = nc.dram_tensor("b2", [HID2], F32, kind="ExternalInput")
    W3 = nc.dram_tensor("W3", [HID2, NUM_CLASSES], F32, kind="ExternalInput")
    b3 = nc.dram_tensor("b3", [NUM_CLASSES], F32, kind="ExternalInput")
    out = nc.dram_tensor("out", [B_LOC, P, NUM_CLASSES], F32, kind="ExternalOutput")

    ident_np, cb_np = _static_consts()
    ident_c = nc.inline_tensor(ident_np, name="c_ident")
    cb_c = nc.inline_tensor(cb_np, name="c_cb")

    with tile.TileContext(nc) as tc:
        emit_kernel(nc, tc, fm[:], prop[:], W1[:], b1[:], W2[:], b2[:], W3[:], b3[:],
                    out[:], (ident_c[:], cb_c[:]))
    nc.compile()
    return nc


_NC_CACHE = None


def _get_module():
    global _NC_CACHE
    if _NC_CACHE is None:
        _NC_CACHE = build_module()
    return _NC_CACHE


def _shard_inputs(inputs):
    f = {k: np.ascontiguousarray(np.asarray(v, dtype=np.float32)) for k, v in inputs.items()}
    in_maps = []
    for c in range(N_CORES):
        sl = slice(B_LOC * c, B_LOC * (c + 1))
        in_maps.append({
            "feature_map": f["feature_map"][sl],
            "proposals": f["proposals"][sl],
            "W1": f["W1"], "b1": f["b1"],
            "W2": f["W2"], "b2": f["b2"],
            "W3": f["W3"], "b3": f["b3"],
        })
    return in_maps


def run(inputs, trace=False):
    """Run on all 8 cores; returns (output [16,22,10], BassKernelResults)."""
    nc = _get_module()
    res = run_bass_kernel_spmd(nc, _shard_inputs(inputs), core_ids=list(range(N_CORES)),
                               trace=trace)
    out = np.concatenate([r["out"] for r in res.results], axis=0)
    return out, res


def kernel(**inputs) -> np.ndarray:
    out, _ = run(inputs, trace=False)
    return out
